# revision 13
# baseline (speedup 1.0000x reference)
"""Trainium2 Bass kernel: 3-layer GCN (AIGEncoder) + global max/sum readout.

8 NeuronCores SPMD. Nodes sharded core = node % 8 (balances per-graph cells
so one compiled schedule serves all cores; per-core structure rides in input
data: index streams + 0/1 reduce matrices). Per layer: bf16 y-table (dinv *
h @ W, node-major) replicated to every core's DRAM (layer 0 built on host
from the 12 distinct embedding rows; layers 1,2 via AllGather); edge
aggregation = chunked dma_gather (edge-major, int16 quarter-local rows) +
TensorEngine 0/1-matrix segment-reduce accumulated in PSUM fp32. ReLU/scales
fold into per-node factors. Readout: gsum via PE graph-mask matmuls, gmax via
DVE max over feature-major h3 (graph-contiguous canonical order); host
combines the 8 partials and rounds.
"""
import sys

sys.path.insert(0, "/opt/trn_rl_repo")

import numpy as np
import ml_dtypes

import concourse.bacc as bacc
import concourse.bass as bass
import concourse.mybir as mybir
from concourse.bass_utils import run_bass_kernel_spmd
from concourse.library_config import mlp

P = 128
N = 100000
NG = 64
D = 128
NC = 8
SHR = None                  # set per-instance: 1 + padded canonical size
CB = 16                     # blocks per chunk
CI = CB * P                 # 2048 idxs per gather


def _host_prep(node_type, ninv, src, dst, batch, emb_type, emb_inv, W0):
    deg = np.bincount(dst, minlength=N) + 1.0
    dinv = (1.0 / np.sqrt(deg)).astype(np.float32)

    cells = np.zeros((NC, NG), dtype=np.int64)
    for c in range(NC):
        cells[c] = np.bincount(batch[np.arange(c, N, NC)], minlength=NG)
    T = cells.max(axis=0)
    cell_start = np.concatenate([[0], np.cumsum(T)])
    ncanon = int(cell_start[-1])
    ntile = -(-ncanon // P)
    ncp = ntile * P

    global SHR
    SHR = ncp + 1
    canon_pos = np.full(N, -1, dtype=np.int64)
    first_of_cell = np.full((NC, NG), -1, dtype=np.int64)
    for c in range(NC):
        nodes_c = np.arange(c, N, NC)
        gs = batch[nodes_c]
        for g in range(NG):
            m = np.flatnonzero(gs == g)
            canon_pos[nodes_c[m]] = cell_start[g] + np.arange(len(m))
            if len(m):
                first_of_cell[c, g] = nodes_c[m[0]]
    trow = (np.arange(N) % NC % 2) * SHR + 1 + canon_pos

    # slot -> (segment rows per quarter); pads duplicate cell's first node
    percore = []
    for c in range(NC):
        slot_node = np.full(ncp, -1, dtype=np.int64)
        nodes_c = np.arange(c, N, NC)
        slot_node[canon_pos[nodes_c]] = nodes_c
        for g in range(NG):
            for j in range(cell_start[g] + cells[c][g], cell_start[g] + T[g]):
                slot_node[j] = first_of_cell[c, g]   # dup (or -1 if empty)
        percore.append(slot_node)

    # per (core, slot) quarter lists
    qs_by_core = []
    for c in range(NC):
        qs = [[[], [], [], []] for _ in range(ncp)]
        sel = np.flatnonzero(dst % NC == c)
        for e in sel:
            s, d = src[e], dst[e]
            qs[canon_pos[d]][(s % NC) // 2].append(int(trow[s]))
        for n in np.arange(c, N, NC):
            qs[canon_pos[n]][(n % NC) // 2].append(int(trow[n]))
        sn = percore[c]
        for j in range(ncp):
            if sn[j] >= 0 and not any(qs[j]):
                # pad slot: duplicate its node's full segment
                nn = sn[j]
                qs[j] = [list(qs[canon_pos[nn]][q]) for q in range(4)]
        qs_by_core.append(qs)

    def need(qs, j):
        return max(len(qs[j][q]) for q in range(4))

    # template blocks per tile
    def blocks_of(qs):
        tiles = []
        for t in range(ntile):
            bl = []
            j = t * P
            while j < (t + 1) * P:
                pos, slots = 0, []
                while j < (t + 1) * P:
                    nd = need(qs, j)
                    if pos + nd > P and slots:
                        break
                    assert pos + nd <= P, "segment too long for one block"
                    slots.append(j)
                    pos += nd
                    j += 1
                bl.append(slots)
            tiles.append(bl)
        return tiles

    all_blocks = [blocks_of(qs_by_core[c]) for c in range(NC)]
    nb_tile = [max(len(all_blocks[c][t]) for c in range(NC)) for t in range(ntile)]
    nblk = sum(nb_tile)
    nblk_pad = -(-nblk // CB) * CB
    nchunk = nblk_pad // CB

    data = []
    for c in range(NC):
        qs = qs_by_core[c]
        idxs = np.zeros((4, nblk_pad * P), dtype=np.int16)
        lhs = np.zeros((nblk_pad, P, P), dtype=ml_dtypes.bfloat16)
        k = 0
        for t in range(ntile):
            bl = all_blocks[c][t]
            for bi in range(nb_tile[t]):
                if bi < len(bl):
                    pos = 0
                    for sj in bl[bi]:
                        nd = need(qs, sj)
                        if nd:
                            lhs[k, pos:pos + nd, sj % P] = 1.0
                            for q in range(4):
                                l = qs[sj][q]
                                idxs[q, k * P + pos:k * P + pos + len(l)] = l
                        pos += nd
                k += 1
        # wrap idx streams into [4, nchunk, P, CI//16]
        wi = np.zeros((4, nchunk, P, CI // 16), dtype=np.int16)
        for q in range(4):
            for ch in range(nchunk):
                part = idxs[q, ch * CI:(ch + 1) * CI]
                w = part.reshape(CI // 16, 16).T
                wi[q, ch] = np.tile(w, (8, 1))
        lhsw = lhs.reshape(nchunk, CB, P, P).transpose(0, 2, 1, 3).reshape(nchunk, P, CB * P).copy()
        data.append(dict(idxs=wi, lhs=lhsw))

    # block k -> tile
    blk_tile = []
    for t in range(ntile):
        blk_tile += [t] * nb_tile[t]
    blk_tile += [ntile - 1] * (nblk_pad - nblk)

    # per-core canonical scalars / masks
    dinvc = np.ones((NC, ncp), dtype=np.float32)
    dinv2 = np.ones((NC, ncp), dtype=np.float32)
    gmask = np.zeros((NC, ncp, NG), dtype=ml_dtypes.bfloat16)
    for c in range(NC):
        nodes_c = np.arange(c, N, NC)
        cp = canon_pos[nodes_c]
        dinvc[c, cp] = dinv[nodes_c]
        dinv2[c, cp] = dinv[nodes_c] ** 2
        gmask[c, cp, batch[nodes_c]] = 1.0

    # layer-0 table: y0[n] = dinv[n] * (emb_type[nt]+emb_inv[ni]) @ W0
    combo = (emb_type[:, None, :] + emb_inv[None, :, :]).reshape(12, D)
    cw = combo @ W0                                  # [12, D]
    y0 = np.zeros((NC, SHR, D), dtype=np.float32)
    cid = node_type * 3 + ninv
    for c in range(NC):
        nodes_c = np.arange(c, N, NC)
        rows = cw[cid[nodes_c]] * dinv[nodes_c][:, None]
        y0[c, 1 + canon_pos[nodes_c] - 0, :] = rows  # canon_pos < SHR-1
    table0 = y0.astype(ml_dtypes.bfloat16)           # [8, SHR, D]

    runs = [(int(cell_start[g]), int(cell_start[g] + T[g])) for g in range(NG)]
    return dict(dinv=dinv, canon_pos=canon_pos, ntile=ntile, ncp=ncp, shr=SHR,
                nb_tile=nb_tile, nblk_pad=nblk_pad, nchunk=nchunk,
                blk_tile=blk_tile, data=data, dinvc=dinvc, dinv2=dinv2,
                gmask=gmask, runs=runs, cells=cells, table0=table0)


def _build(tpl):
    global SHR
    SHR = tpl["shr"]
    ntile, nchunk, nblk_pad = tpl["ntile"], tpl["nchunk"], tpl["nblk_pad"]
    nb_tile, blk_tile, runs = tpl["nb_tile"], tpl["blk_tile"], tpl["runs"]
    assert len(runs) == NG
    dt = mybir.dt
    f32, bf16, i16 = dt.float32, dt.bfloat16, dt.int16

    nc = bacc.Bacc("TRN2", debug=False, num_swdge_queues=2, num_devices=NC)
    w1_d = nc.dram_tensor("w1b", [D, D], bf16, kind="ExternalInput")
    w2_d = nc.dram_tensor("w2b", [D, D], bf16, kind="ExternalInput")
    t0_d = nc.dram_tensor("table0", [NC, SHR, D], bf16, kind="ExternalInput")
    idxs_d = nc.dram_tensor("idxs", [4, nchunk, P, CI // 16], i16, kind="ExternalInput")
    lhs_d = nc.dram_tensor("lhs", [nchunk, P, CB * P], bf16, kind="ExternalInput")
    dinv2_d = nc.dram_tensor("dinv2", [ntile, P], f32, kind="ExternalInput")
    dinvc_d = nc.dram_tensor("dinvc", [ntile, P], f32, kind="ExternalInput")
    gmask_d = nc.dram_tensor("gmask", [ntile, P, NG], bf16, kind="ExternalInput")
    ident_d = nc.dram_tensor("ident", [P, P], bf16, kind="ExternalInput")
    gsum_o = nc.dram_tensor("gsum", [NG, D], f32, kind="ExternalOutput")
    gmax_o = nc.dram_tensor("gmax", [P, NG], f32, kind="ExternalOutput")

    ystage = nc.dram_tensor("ystage", [SHR, D], bf16)
    tables = [t0_d] + [nc.dram_tensor(f"table{l}", [NC, SHR, D], bf16,
                                      addr_space="Shared") for l in (1, 2)]

    tile_k0 = np.cumsum([0] + nb_tile)[:-1]
    tile_k1 = tile_k0 + np.array(nb_tile) - 1
    tile_k1[-1] = nblk_pad - 1

    # ---- compile-time op numbering ----
    mm_idx, ev_idx, vfm_idx, tr_idx, xw_idx, ysc_idx = {}, {}, {}, {}, {}, {}
    pe_n = act_n = dve_n = 0
    for l in range(3):
        for k in range(nblk_pad):
            for q in range(4):
                pe_n += 1
                mm_idx[(l, k, q)] = pe_n
        for t in range(ntile):
            act_n += 1
            ev_idx[(l, t)] = act_n
        for t in range(ntile):
            pe_n += 1
            tr_idx[(l, t)] = pe_n
            act_n += 1
            vfm_idx[(l, t)] = act_n
        for t in range(ntile):
            pe_n += 1
            xw_idx[(l, t)] = pe_n          # l==2: gsum matmuls
            if l < 2:
                dve_n += 1
                ysc_idx[(l, t)] = dve_n

    n_init = 7
    sd_chunk = {}
    SD_Y = {}
    sd = n_init
    for l in range(3):
        for ch in range(nchunk):
            sd += 5
            sd_chunk[(l, ch)] = sd * 16
        if l < 2:
            sd += 1
            SD_Y[l] = sd * 16
    SD_TOTAL = (sd + 2) * 16

    gthr = {}
    cnt = {}
    for l in range(3):
        for ch in range(nchunk):
            for q in range(4):
                key = (q, ch % 2)
                cnt[key] = cnt.get(key, 0) + 16
                gthr[(l, ch, q)] = cnt[key]

    from contextlib import ExitStack
    es = ExitStack()
    block = es.enter_context(nc.Block())
    sb = lambda *a: es.enter_context(nc.sbuf_tensor(*a))
    ps = lambda *a: es.enter_context(nc.psum_tensor(*a))
    sem = lambda n: es.enter_context(nc.semaphore(n))
    bf16_, f32_, i16_ = bf16, f32, i16
    msgs = sb("msgs", [P, 4, 2, CB, D], bf16)
    idx_sb = sb("idx_sb", [P, 4, 2, CI // 16], i16)
    lhs_sb = sb("lhs_sb", [P, 2, CB, P], bf16)
    vnm = sb("vnm", [P, ntile, D], bf16)
    vfm = sb("vfm", [P, ntile, P], bf16)
    ynm = sb("ynm", [P, ntile, D], bf16)
    w_sb = sb("w_sb", [P, 2, D], bf16)
    dinv2_sb = sb("dinv2_sb", [P, ntile], f32)
    dinvc_sb = sb("dinvc_sb", [P, ntile], f32)
    gmask_sb = sb("gmask_sb", [P, ntile, NG], bf16)
    ident_sb = sb("ident_sb", [P, P], bf16)
    gmax_sb = sb("gmax_sb", [P, NG], f32)
    gsum_sb = sb("gsum_sb", [NG, D], f32)
    zrow = sb("zrow", [1, D], bf16)
    ps_red = ps("ps_red", [P, 4, 512], f32)
    ps_aux = ps("ps_aux", [P, 2, 1024], bf16)
    ps_gs = ps("ps_gs", [NG, D], f32)
    sdma = sem("sdma")
    gsem = [[sem(f"g{q}{p}") for p in range(2)] for q in range(4)]
    pe_s = sem("pe")
    act_s = sem("act")
    dve_s = sem("dve")
    cc_s = sem("cc")
    if True:

        @block.sync
        def _(s):
            _anc = nc.allow_non_contiguous_dma(reason="tiny one-time scale loads")
            _anc.__enter__()
            s.dma_start(w_sb[:, 0, :], w1_d[:]).then_inc(sdma, 16)
            s.dma_start(w_sb[:, 1, :], w2_d[:]).then_inc(sdma, 16)
            s.dma_start(dinv2_sb[:], dinv2_d[:].rearrange("t p -> p t")).then_inc(sdma, 16)
            s.dma_start(dinvc_sb[:], dinvc_d[:].rearrange("t p -> p t")).then_inc(sdma, 16)
            s.dma_start(gmask_sb[:], gmask_d[:].rearrange("t p g -> p t g")).then_inc(sdma, 16)
            s.dma_start(ident_sb[:], ident_d[:]).then_inc(sdma, 16)
            s.wait_ge(dve_s, 1)
            s.dma_start(ystage[0:1, :], zrow[:]).then_inc(sdma, 16)
            _anc.__exit__(None, None, None)
            for l in range(3):
                for ch in range(nchunk):
                    par = ch % 2
                    if ch >= 2:
                        # buffers free when PE consumed chunk ch-2 (this layer)
                        k_last = (ch - 2) * CB + CB - 1
                        s.wait_ge(pe_s, mm_idx[(l, k_last, 3)])
                    for q in range(4):
                        s.dma_start(idx_sb[:, q, par, :], idxs_d[q, ch]).then_inc(sdma, 16)
                    s.dma_start(lhs_sb[:, par, :, :], lhs_d[ch].rearrange("p (b m) -> p b m", m=P)).then_inc(sdma, 16)
                if l < 2:
                    s.wait_ge(dve_s, 1 + ysc_idx[(l, ntile - 1)])
                    s.dma_start(ystage[1:, :].rearrange("(t p) f -> p t f", p=P), ynm[:]).then_inc(sdma, 16)
            # outputs
            s.wait_ge(dve_s, 1 + dve_n + 1 + NG)          # gmax done (memset + runs)
            s.wait_ge(act_s, act_n + 1)               # gsum evac done
            s.dma_start(gsum_o[:], gsum_sb[:]).then_inc(sdma, 16)
            s.dma_start(gmax_o[:], gmax_sb[:]).then_inc(sdma, 16)
            s.wait_ge(sdma, SD_TOTAL)

        @block.gpsimd
        def _(gp):
            gp.load_library(mlp)
            for l in range(3):
                for ch in range(nchunk):
                    par = ch % 2
                    for q in range(4):
                        if ch >= 2:
                            k_last = (ch - 2) * CB + CB - 1
                            gp.wait_ge(pe_s, mm_idx[(l, k_last, 3)])
                        gp.wait_ge(sdma, sd_chunk[(l, ch)])
                        if l > 0:
                            gp.wait_ge(cc_s, l)
                        gp.dma_gather(
                            msgs[:, q, par, :, :],
                            tables[l][2 * q:2 * q + 2].rearrange("a b c -> (a b) c"),
                            idx_sb[:, q, par, :],
                            CI, CI, D,
                            transpose=False,
                            single_packet=False,
                            queue_num=q % 2,
                        ).then_inc(gsem[q][par], 16)
                if l < 2:
                    gp.wait_ge(sdma, SD_Y[l])
                    gp.collective_compute(
                        "AllGather", mybir.AluOpType.bypass,
                        ins=[ystage[:]], outs=[tables[l + 1][:]],
                        replica_groups=[list(range(NC))],
                    ).then_inc(cc_s, 1)

        @block.tensor
        def _(pe):
            for l in range(3):
                for k in range(nblk_pad):
                    t = blk_tile[k]
                    ch, par = k // CB, (k // CB) % 2
                    for q in range(4):
                        if q == 0 and k % CB == 0:
                            for qq in range(4):
                                pe.wait_ge(gsem[qq][par], gthr[(l, ch, qq)])
                            pe.wait_ge(sdma, sd_chunk[(l, ch)])
                        if k == tile_k0[t] and q == 0 and t >= 4:
                            pe.wait_ge(act_s, ev_idx[(l, t - 4)] if t - 4 >= 0 else 0)
                        pe.matmul(
                            out=ps_red[:, t % 4, :D],
                            lhsT=lhs_sb[:, par, k % CB, :],
                            rhs=msgs[:, q, par, k % CB, :],
                            start=(k == tile_k0[t] and q == 0),
                            stop=(k == tile_k1[t] and q == 3),
                        ).then_inc(pe_s, 1)
                for t in range(ntile):
                    pe.wait_ge(act_s, ev_idx[(l, t)])
                    if t >= 2:
                        pe.wait_ge(act_s, vfm_idx[(l, t - 2)])
                    pe.transpose(out=ps_aux[:, t % 2, :P], in_=vnm[:, t, :],
                                 identity=ident_sb[:]).then_inc(pe_s, 1)
                if l < 2:
                    for t in range(ntile):
                        pe.wait_ge(act_s, vfm_idx[(l, t)])
                        if t >= 4:
                            pe.wait_ge(dve_s, 1 + ysc_idx[(l, t - 4)])
                        pe.matmul(out=ps_red[:, t % 4, :D], lhsT=vfm[:, t, :],
                                  rhs=w_sb[:, l, :], start=True, stop=True,
                                  ).then_inc(pe_s, 1)
                else:
                    for t in range(ntile):
                        pe.wait_ge(act_s, ev_idx[(2, t)])
                        pe.matmul(out=ps_gs[:], lhsT=gmask_sb[:, t, :],
                                  rhs=vnm[:, t, :], start=(t == 0),
                                  stop=(t == ntile - 1)).then_inc(pe_s, 1)

        @block.scalar
        def _(a):
            Act = mybir.ActivationFunctionType
            for l in range(3):
                for t in range(ntile):
                    a.wait_ge(pe_s, mm_idx[(l, int(tile_k1[t]), 3)])
                    if l < 2:
                        a.activation(vnm[:, t, :], ps_red[:, t % 4, :D], Act.Relu).then_inc(act_s, 1)
                    else:
                        a.activation(vnm[:, t, :], ps_red[:, t % 4, :D], Act.Copy,
                                     scale=dinvc_sb[:, t:t + 1]).then_inc(act_s, 1)
                for t in range(ntile):
                    a.wait_ge(pe_s, tr_idx[(l, t)])
                    a.activation(vfm[:, t, :], ps_aux[:, t % 2, :P], Act.Copy).then_inc(act_s, 1)
            a.wait_ge(pe_s, xw_idx[(2, ntile - 1)])
            a.activation(gsum_sb[:], ps_gs[:], Act.Copy).then_inc(act_s, 1)

        @block.vector
        def _(v):
            v.memset(zrow[:], 0.0).then_inc(dve_s, 1)
            for l in range(2):
                for t in range(ntile):
                    v.wait_ge(pe_s, xw_idx[(l, t)])
                    v.tensor_scalar(out=ynm[:, t, :], in0=ps_red[:, t % 4, :D],
                                    scalar1=dinv2_sb[:, t:t + 1], scalar2=None,
                                    op0=mybir.AluOpType.mult).then_inc(dve_s, 1)
            v.memset(gmax_sb[:], -1e30).then_inc(dve_s, 1)
            vf = vfm[:].rearrange("p t f -> p (t f)")
            for g in range(NG):
                a, b = runs[g]
                v.wait_ge(act_s, vfm_idx[(2, (b - 1) // P)])
                v.tensor_reduce(out=gmax_sb[:, g:g + 1], in_=vf[:, a:b],
                                axis=mybir.AxisListType.X,
                                op=mybir.AluOpType.max).then_inc(dve_s, 1)

    es.close()
    nc.compile()
    return nc


def kernel(**inputs):
    node_type = np.asarray(inputs["node_type"]).astype(np.int64)
    ninv = np.asarray(inputs["num_inverted_predecessors"]).astype(np.int64)
    ei = np.asarray(inputs["edge_index"]).astype(np.int64)
    batch = np.asarray(inputs["batch"]).astype(np.int64)
    emb_type = np.asarray(inputs["emb_type"]).astype(np.float32)
    emb_inv = np.asarray(inputs["emb_inv"]).astype(np.float32)
    W0 = np.asarray(inputs["W0"]).astype(np.float32)
    W1 = np.asarray(inputs["W1"]).astype(np.float32)
    W2 = np.asarray(inputs["W2"]).astype(np.float32)

    tpl = _host_prep(node_type, ninv, ei[0], ei[1], batch, emb_type, emb_inv, W0)
    nc = _build(tpl)

    ident = np.eye(P, dtype=ml_dtypes.bfloat16)
    in_maps = []
    for c in range(NC):
        in_maps.append(dict(
            w1b=W1.astype(ml_dtypes.bfloat16),
            w2b=W2.astype(ml_dtypes.bfloat16),
            table0=tpl["table0"],
            idxs=tpl["data"][c]["idxs"],
            lhs=tpl["data"][c]["lhs"],
            dinv2=tpl["dinv2"][c].reshape(tpl["ntile"], P),
            dinvc=tpl["dinvc"][c].reshape(tpl["ntile"], P),
            gmask=tpl["gmask"][c].reshape(tpl["ntile"], P, NG),
            ident=ident,
        ))
    import os
    trace = os.environ.get("BASS_KERNEL_TRACE", "0") == "1"
    if trace:
        sys.path.insert(0, "/root/problem/work")
        try:
            import axon_trace_patch  # noqa
        except Exception:
            trace = False
    res = run_bass_kernel_spmd(nc, in_maps, core_ids=list(range(NC)), trace=trace)
    kernel.last_exec_ns = res.exec_time_ns

    gsum = np.zeros((NG, D), dtype=np.float64)
    gmax = np.full((NG, D), -np.inf)
    for c in range(NC):
        gsum += res.results[c]["gsum"].astype(np.float64)
        gm = res.results[c]["gmax"].astype(np.float64).T   # [NG? no: [P,NG]->T = [NG,P]]
        pres = tpl["cells"][c] > 0
        gmax[pres] = np.maximum(gmax[pres], gm[pres])
    out = np.concatenate([gmax, gsum], axis=1).astype(np.float32)
    return (np.round(out * 1000.0) / 1000.0).astype(np.float32)


# revision 14
# speedup vs baseline: 1.1154x; 1.1154x over previous
"""Trainium2 Bass kernel: 3-layer GCN (AIGEncoder) + global max/sum readout.

8 NeuronCores SPMD. Nodes sharded core = node % 8 (balances per-graph cells
so one compiled schedule serves all cores; per-core structure rides in input
data: index streams + 0/1 reduce matrices). Per layer: bf16 y-table (dinv *
h @ W, node-major) replicated to every core's DRAM (layer 0 built on host
from the 12 distinct embedding rows; layers 1,2 via AllGather); edge
aggregation = chunked dma_gather (edge-major, int16 quarter-local rows) +
TensorEngine 0/1-matrix segment-reduce accumulated in PSUM fp32. ReLU/scales
fold into per-node factors. Readout: gsum via PE graph-mask matmuls, gmax via
DVE max over feature-major h3 (graph-contiguous canonical order); host
combines the 8 partials and rounds.
"""
import sys

sys.path.insert(0, "/opt/trn_rl_repo")

import numpy as np
import ml_dtypes

import concourse.bacc as bacc
import concourse.bass as bass
import concourse.mybir as mybir
from concourse.bass_utils import run_bass_kernel_spmd
from concourse.library_config import mlp

P = 128
N = 100000
NG = 64
D = 128
NC = 8
SHR = None                  # set per-instance: 1 + padded canonical size
CB = 16                     # blocks per chunk
CI = CB * P                 # 2048 idxs per gather
NBUFC = 3                   # chunk pipeline depth


def _host_prep(node_type, ninv, src, dst, batch, emb_type, emb_inv, W0):
    deg = np.bincount(dst, minlength=N) + 1.0
    dinv = (1.0 / np.sqrt(deg)).astype(np.float32)

    cells = np.zeros((NC, NG), dtype=np.int64)
    for c in range(NC):
        cells[c] = np.bincount(batch[np.arange(c, N, NC)], minlength=NG)
    T = cells.max(axis=0)
    cell_start = np.concatenate([[0], np.cumsum(T)])
    ncanon = int(cell_start[-1])
    ntile = -(-ncanon // P)
    ncp = ntile * P

    global SHR
    SHR = ncp + 1
    canon_pos = np.full(N, -1, dtype=np.int64)
    first_of_cell = np.full((NC, NG), -1, dtype=np.int64)
    for c in range(NC):
        nodes_c = np.arange(c, N, NC)
        gs = batch[nodes_c]
        for g in range(NG):
            m = np.flatnonzero(gs == g)
            canon_pos[nodes_c[m]] = cell_start[g] + np.arange(len(m))
            if len(m):
                first_of_cell[c, g] = nodes_c[m[0]]
    trow = (np.arange(N) % NC % 2) * SHR + 1 + canon_pos

    # slot -> (segment rows per quarter); pads duplicate cell's first node
    percore = []
    for c in range(NC):
        slot_node = np.full(ncp, -1, dtype=np.int64)
        nodes_c = np.arange(c, N, NC)
        slot_node[canon_pos[nodes_c]] = nodes_c
        for g in range(NG):
            for j in range(cell_start[g] + cells[c][g], cell_start[g] + T[g]):
                slot_node[j] = first_of_cell[c, g]   # dup (or -1 if empty)
        percore.append(slot_node)

    # per (core, slot) quarter lists
    qs_by_core = []
    for c in range(NC):
        qs = [[[], [], [], []] for _ in range(ncp)]
        sel = np.flatnonzero(dst % NC == c)
        for e in sel:
            s, d = src[e], dst[e]
            qs[canon_pos[d]][(s % NC) // 2].append(int(trow[s]))
        for n in np.arange(c, N, NC):
            qs[canon_pos[n]][(n % NC) // 2].append(int(trow[n]))
        sn = percore[c]
        for j in range(ncp):
            if sn[j] >= 0 and not any(qs[j]):
                # pad slot: duplicate its node's full segment
                nn = sn[j]
                qs[j] = [list(qs[canon_pos[nn]][q]) for q in range(4)]
        qs_by_core.append(qs)

    def need(qs, j):
        return max(len(qs[j][q]) for q in range(4))

    # template blocks per tile
    def blocks_of(qs):
        tiles = []
        for t in range(ntile):
            bl = []
            j = t * P
            while j < (t + 1) * P:
                pos, slots = 0, []
                while j < (t + 1) * P:
                    nd = need(qs, j)
                    if pos + nd > P and slots:
                        break
                    assert pos + nd <= P, "segment too long for one block"
                    slots.append(j)
                    pos += nd
                    j += 1
                bl.append(slots)
            tiles.append(bl)
        return tiles

    all_blocks = [blocks_of(qs_by_core[c]) for c in range(NC)]
    nb_tile = [max(len(all_blocks[c][t]) for c in range(NC)) for t in range(ntile)]
    nblk = sum(nb_tile)
    nblk_pad = -(-nblk // CB) * CB
    nchunk = nblk_pad // CB

    data = []
    for c in range(NC):
        qs = qs_by_core[c]
        idxs = np.zeros((4, nblk_pad * P), dtype=np.int16)
        lhs = np.zeros((nblk_pad, P, P), dtype=ml_dtypes.bfloat16)
        k = 0
        for t in range(ntile):
            bl = all_blocks[c][t]
            for bi in range(nb_tile[t]):
                if bi < len(bl):
                    pos = 0
                    for sj in bl[bi]:
                        nd = need(qs, sj)
                        if nd:
                            lhs[k, pos:pos + nd, sj % P] = 1.0
                            for q in range(4):
                                l = qs[sj][q]
                                idxs[q, k * P + pos:k * P + pos + len(l)] = l
                        pos += nd
                k += 1
        # wrap idx streams into [4, nchunk, P, CI//16]
        wi = np.zeros((4, nchunk, P, CI // 16), dtype=np.int16)
        for q in range(4):
            for ch in range(nchunk):
                part = idxs[q, ch * CI:(ch + 1) * CI]
                w = part.reshape(CI // 16, 16).T
                wi[q, ch] = np.tile(w, (8, 1))
        lhsw = lhs.reshape(nchunk, CB, P, P).transpose(0, 2, 1, 3).reshape(nchunk, P, CB * P).copy()
        data.append(dict(idxs=wi, lhs=lhsw))

    # block k -> tile
    blk_tile = []
    for t in range(ntile):
        blk_tile += [t] * nb_tile[t]
    blk_tile += [ntile - 1] * (nblk_pad - nblk)

    # per-core canonical scalars / masks
    dinvc = np.ones((NC, ncp), dtype=np.float32)
    dinv2 = np.ones((NC, ncp), dtype=np.float32)
    gmask = np.zeros((NC, ncp, NG), dtype=ml_dtypes.bfloat16)
    for c in range(NC):
        nodes_c = np.arange(c, N, NC)
        cp = canon_pos[nodes_c]
        dinvc[c, cp] = dinv[nodes_c]
        dinv2[c, cp] = dinv[nodes_c] ** 2
        gmask[c, cp, batch[nodes_c]] = 1.0

    # layer-0 table: y0[n] = dinv[n] * (emb_type[nt]+emb_inv[ni]) @ W0
    combo = (emb_type[:, None, :] + emb_inv[None, :, :]).reshape(12, D)
    cw = combo @ W0                                  # [12, D]
    y0 = np.zeros((NC, SHR, D), dtype=np.float32)
    cid = node_type * 3 + ninv
    for c in range(NC):
        nodes_c = np.arange(c, N, NC)
        rows = cw[cid[nodes_c]] * dinv[nodes_c][:, None]
        y0[c, 1 + canon_pos[nodes_c] - 0, :] = rows  # canon_pos < SHR-1
    table0 = y0.astype(ml_dtypes.bfloat16)           # [8, SHR, D]

    runs = [(int(cell_start[g]), int(cell_start[g] + T[g])) for g in range(NG)]
    return dict(dinv=dinv, canon_pos=canon_pos, ntile=ntile, ncp=ncp, shr=SHR,
                nb_tile=nb_tile, nblk_pad=nblk_pad, nchunk=nchunk,
                blk_tile=blk_tile, data=data, dinvc=dinvc, dinv2=dinv2,
                gmask=gmask, runs=runs, cells=cells, table0=table0)


def _build(tpl):
    global SHR
    SHR = tpl["shr"]
    ntile, nchunk, nblk_pad = tpl["ntile"], tpl["nchunk"], tpl["nblk_pad"]
    nb_tile, blk_tile, runs = tpl["nb_tile"], tpl["blk_tile"], tpl["runs"]
    assert len(runs) == NG
    dt = mybir.dt
    f32, bf16, i16 = dt.float32, dt.bfloat16, dt.int16

    nc = bacc.Bacc("TRN2", debug=False, num_swdge_queues=2, num_devices=NC)
    w1_d = nc.dram_tensor("w1b", [D, D], bf16, kind="ExternalInput")
    w2_d = nc.dram_tensor("w2b", [D, D], bf16, kind="ExternalInput")
    t0_d = nc.dram_tensor("table0", [NC, SHR, D], bf16, kind="ExternalInput")
    idxs_d = nc.dram_tensor("idxs", [4, nchunk, P, CI // 16], i16, kind="ExternalInput")
    lhs_d = nc.dram_tensor("lhs", [nchunk, P, CB * P], bf16, kind="ExternalInput")
    dinv2_d = nc.dram_tensor("dinv2", [ntile, P], f32, kind="ExternalInput")
    dinvc_d = nc.dram_tensor("dinvc", [ntile, P], f32, kind="ExternalInput")
    gmask_d = nc.dram_tensor("gmask", [ntile, P, NG], bf16, kind="ExternalInput")
    ident_d = nc.dram_tensor("ident", [P, P], bf16, kind="ExternalInput")
    gsum_o = nc.dram_tensor("gsum", [NG, D], f32, kind="ExternalOutput")
    gmax_o = nc.dram_tensor("gmax", [P, NG], f32, kind="ExternalOutput")

    ystage = nc.dram_tensor("ystage", [SHR, D], bf16)
    tables = [t0_d] + [nc.dram_tensor(f"table{l}", [NC, SHR, D], bf16,
                                      addr_space="Shared") for l in (1, 2)]

    tile_k0 = np.cumsum([0] + nb_tile)[:-1]
    tile_k1 = tile_k0 + np.array(nb_tile) - 1
    tile_k1[-1] = nblk_pad - 1

    # ---- compile-time op numbering ----
    mm_idx, ev_idx, vfm_idx, tr_idx, xw_idx, ysc_idx = {}, {}, {}, {}, {}, {}
    pe_n = act_n = dve_n = 0
    for l in range(3):
        for k in range(nblk_pad):
            for q in range(4):
                pe_n += 1
                mm_idx[(l, k, q)] = pe_n
        for t in range(ntile):
            act_n += 1
            ev_idx[(l, t)] = act_n
        for t in range(ntile):
            pe_n += 1
            tr_idx[(l, t)] = pe_n
            act_n += 1
            vfm_idx[(l, t)] = act_n
        for t in range(ntile):
            pe_n += 1
            xw_idx[(l, t)] = pe_n          # l==2: gsum matmuls
            if l < 2:
                dve_n += 1
                ysc_idx[(l, t)] = dve_n

    n_init = 7
    sd_chunk = {}
    SD_Y = {}
    sd = n_init
    for l in range(3):
        for ch in range(nchunk):
            sd += 5
            sd_chunk[(l, ch)] = sd * 16
        if l < 2:
            sd += 1
            SD_Y[l] = sd * 16
    SD_TOTAL = (sd + 2) * 16

    gthr = {}
    cnt = {}
    for l in range(3):
        for ch in range(nchunk):
            for q in range(4):
                key = (q, ch % NBUFC)
                cnt[key] = cnt.get(key, 0) + 16
                gthr[(l, ch, q)] = cnt[key]

    from contextlib import ExitStack
    es = ExitStack()
    block = es.enter_context(nc.Block())
    sb = lambda *a: es.enter_context(nc.sbuf_tensor(*a))
    ps = lambda *a: es.enter_context(nc.psum_tensor(*a))
    sem = lambda n: es.enter_context(nc.semaphore(n))
    bf16_, f32_, i16_ = bf16, f32, i16
    msgs = sb("msgs", [P, 4, NBUFC, CB, D], bf16)
    idx_sb = sb("idx_sb", [P, 4, NBUFC, CI // 16], i16)
    lhs_sb = sb("lhs_sb", [P, NBUFC, CB, P], bf16)
    vnm = sb("vnm", [P, ntile, D], bf16)
    vfm = sb("vfm", [P, ntile, P], bf16)
    ynm = sb("ynm", [P, ntile, D], bf16)
    w_sb = sb("w_sb", [P, 2, D], bf16)
    dinv2_sb = sb("dinv2_sb", [P, ntile], f32)
    dinvc_sb = sb("dinvc_sb", [P, ntile], f32)
    gmask_sb = sb("gmask_sb", [P, ntile, NG], bf16)
    ident_sb = sb("ident_sb", [P, P], bf16)
    gmax_sb = sb("gmax_sb", [P, NG], f32)
    gsum_sb = sb("gsum_sb", [NG, D], f32)
    zrow = sb("zrow", [1, D], bf16)
    ps_red = ps("ps_red", [P, 4, 512], f32)
    ps_aux = ps("ps_aux", [P, 2, 1024], bf16)
    ps_gs = ps("ps_gs", [NG, D], f32)
    sdma = sem("sdma")
    gsem = [[sem(f"g{q}{p}") for p in range(NBUFC)] for q in range(4)]
    pe_s = sem("pe")
    act_s = sem("act")
    dve_s = sem("dve")
    cc_s = sem("cc")
    if True:

        @block.sync
        def _(s):
            _anc = nc.allow_non_contiguous_dma(reason="tiny one-time scale loads")
            _anc.__enter__()
            s.dma_start(w_sb[:, 0, :], w1_d[:]).then_inc(sdma, 16)
            s.dma_start(w_sb[:, 1, :], w2_d[:]).then_inc(sdma, 16)
            s.dma_start(dinv2_sb[:], dinv2_d[:].rearrange("t p -> p t")).then_inc(sdma, 16)
            s.dma_start(dinvc_sb[:], dinvc_d[:].rearrange("t p -> p t")).then_inc(sdma, 16)
            s.dma_start(gmask_sb[:], gmask_d[:].rearrange("t p g -> p t g")).then_inc(sdma, 16)
            s.dma_start(ident_sb[:], ident_d[:]).then_inc(sdma, 16)
            s.wait_ge(dve_s, 1)
            s.dma_start(ystage[0:1, :], zrow[:]).then_inc(sdma, 16)
            _anc.__exit__(None, None, None)
            for l in range(3):
                for ch in range(nchunk):
                    par = ch % NBUFC
                    if ch >= NBUFC:
                        # buffers free when PE consumed chunk ch-NBUFC
                        k_last = (ch - NBUFC) * CB + CB - 1
                        s.wait_ge(pe_s, mm_idx[(l, k_last, 3)])
                    for q in range(4):
                        s.dma_start(idx_sb[:, q, par, :], idxs_d[q, ch]).then_inc(sdma, 16)
                    s.dma_start(lhs_sb[:, par, :, :], lhs_d[ch].rearrange("p (b m) -> p b m", m=P)).then_inc(sdma, 16)
                if l < 2:
                    s.wait_ge(dve_s, 1 + ysc_idx[(l, ntile - 1)])
                    s.dma_start(ystage[1:, :].rearrange("(t p) f -> p t f", p=P), ynm[:]).then_inc(sdma, 16)
            # outputs
            s.wait_ge(dve_s, 1 + dve_n + 1 + NG)          # gmax done (memset + runs)
            s.wait_ge(act_s, act_n + 1)               # gsum evac done
            s.dma_start(gsum_o[:], gsum_sb[:]).then_inc(sdma, 16)
            s.dma_start(gmax_o[:], gmax_sb[:]).then_inc(sdma, 16)
            s.wait_ge(sdma, SD_TOTAL)

        @block.gpsimd
        def _(gp):
            gp.load_library(mlp)
            for l in range(3):
                for ch in range(nchunk):
                    par = ch % NBUFC
                    for q in range(4):
                        if ch >= NBUFC:
                            k_last = (ch - NBUFC) * CB + CB - 1
                            gp.wait_ge(pe_s, mm_idx[(l, k_last, 3)])
                        gp.wait_ge(sdma, sd_chunk[(l, ch)])
                        if l > 0:
                            gp.wait_ge(cc_s, l)
                        gp.dma_gather(
                            msgs[:, q, par, :, :],
                            tables[l][2 * q:2 * q + 2].rearrange("a b c -> (a b) c"),
                            idx_sb[:, q, par, :],
                            CI, CI, D,
                            transpose=False,
                            single_packet=False,
                            queue_num=q % 2,
                        ).then_inc(gsem[q][par], 16)
                if l < 2:
                    gp.wait_ge(sdma, SD_Y[l])
                    gp.collective_compute(
                        "AllGather", mybir.AluOpType.bypass,
                        ins=[ystage[:]], outs=[tables[l + 1][:]],
                        replica_groups=[list(range(NC))],
                    ).then_inc(cc_s, 1)

        @block.tensor
        def _(pe):
            for l in range(3):
                for k in range(nblk_pad):
                    t = blk_tile[k]
                    ch, par = k // CB, (k // CB) % NBUFC
                    for q in range(4):
                        if q == 0 and k % CB == 0:
                            for qq in range(4):
                                pe.wait_ge(gsem[qq][par], gthr[(l, ch, qq)])
                            pe.wait_ge(sdma, sd_chunk[(l, ch)])
                        if k == tile_k0[t] and q == 0 and t >= 4:
                            pe.wait_ge(act_s, ev_idx[(l, t - 4)] if t - 4 >= 0 else 0)
                        pe.matmul(
                            out=ps_red[:, t % 4, :D],
                            lhsT=lhs_sb[:, par, k % CB, :],
                            rhs=msgs[:, q, par, k % CB, :],
                            start=(k == tile_k0[t] and q == 0),
                            stop=(k == tile_k1[t] and q == 3),
                        ).then_inc(pe_s, 1)
                for t in range(ntile):
                    pe.wait_ge(act_s, ev_idx[(l, t)])
                    if t >= 2:
                        pe.wait_ge(act_s, vfm_idx[(l, t - 2)])
                    pe.transpose(out=ps_aux[:, t % 2, :P], in_=vnm[:, t, :],
                                 identity=ident_sb[:]).then_inc(pe_s, 1)
                if l < 2:
                    for t in range(ntile):
                        pe.wait_ge(act_s, vfm_idx[(l, t)])
                        if t >= 4:
                            pe.wait_ge(dve_s, 1 + ysc_idx[(l, t - 4)])
                        pe.matmul(out=ps_red[:, t % 4, :D], lhsT=vfm[:, t, :],
                                  rhs=w_sb[:, l, :], start=True, stop=True,
                                  ).then_inc(pe_s, 1)
                else:
                    for t in range(ntile):
                        pe.wait_ge(act_s, ev_idx[(2, t)])
                        pe.matmul(out=ps_gs[:], lhsT=gmask_sb[:, t, :],
                                  rhs=vnm[:, t, :], start=(t == 0),
                                  stop=(t == ntile - 1)).then_inc(pe_s, 1)

        @block.scalar
        def _(a):
            Act = mybir.ActivationFunctionType
            for l in range(3):
                for t in range(ntile):
                    a.wait_ge(pe_s, mm_idx[(l, int(tile_k1[t]), 3)])
                    if l < 2:
                        a.activation(vnm[:, t, :], ps_red[:, t % 4, :D], Act.Relu).then_inc(act_s, 1)
                    else:
                        a.activation(vnm[:, t, :], ps_red[:, t % 4, :D], Act.Copy,
                                     scale=dinvc_sb[:, t:t + 1]).then_inc(act_s, 1)
                for t in range(ntile):
                    a.wait_ge(pe_s, tr_idx[(l, t)])
                    a.activation(vfm[:, t, :], ps_aux[:, t % 2, :P], Act.Copy).then_inc(act_s, 1)
            a.wait_ge(pe_s, xw_idx[(2, ntile - 1)])
            a.activation(gsum_sb[:], ps_gs[:], Act.Copy).then_inc(act_s, 1)

        @block.vector
        def _(v):
            v.memset(zrow[:], 0.0).then_inc(dve_s, 1)
            for l in range(2):
                for t in range(ntile):
                    v.wait_ge(pe_s, xw_idx[(l, t)])
                    v.tensor_scalar(out=ynm[:, t, :], in0=ps_red[:, t % 4, :D],
                                    scalar1=dinv2_sb[:, t:t + 1], scalar2=None,
                                    op0=mybir.AluOpType.mult).then_inc(dve_s, 1)
            v.memset(gmax_sb[:], -1e30).then_inc(dve_s, 1)
            vf = vfm[:].rearrange("p t f -> p (t f)")
            for g in range(NG):
                a, b = runs[g]
                v.wait_ge(act_s, vfm_idx[(2, (b - 1) // P)])
                v.tensor_reduce(out=gmax_sb[:, g:g + 1], in_=vf[:, a:b],
                                axis=mybir.AxisListType.X,
                                op=mybir.AluOpType.max).then_inc(dve_s, 1)

    es.close()
    nc.compile()
    return nc


def kernel(**inputs):
    node_type = np.asarray(inputs["node_type"]).astype(np.int64)
    ninv = np.asarray(inputs["num_inverted_predecessors"]).astype(np.int64)
    ei = np.asarray(inputs["edge_index"]).astype(np.int64)
    batch = np.asarray(inputs["batch"]).astype(np.int64)
    emb_type = np.asarray(inputs["emb_type"]).astype(np.float32)
    emb_inv = np.asarray(inputs["emb_inv"]).astype(np.float32)
    W0 = np.asarray(inputs["W0"]).astype(np.float32)
    W1 = np.asarray(inputs["W1"]).astype(np.float32)
    W2 = np.asarray(inputs["W2"]).astype(np.float32)

    tpl = _host_prep(node_type, ninv, ei[0], ei[1], batch, emb_type, emb_inv, W0)
    nc = _build(tpl)

    ident = np.eye(P, dtype=ml_dtypes.bfloat16)
    in_maps = []
    for c in range(NC):
        in_maps.append(dict(
            w1b=W1.astype(ml_dtypes.bfloat16),
            w2b=W2.astype(ml_dtypes.bfloat16),
            table0=tpl["table0"],
            idxs=tpl["data"][c]["idxs"],
            lhs=tpl["data"][c]["lhs"],
            dinv2=tpl["dinv2"][c].reshape(tpl["ntile"], P),
            dinvc=tpl["dinvc"][c].reshape(tpl["ntile"], P),
            gmask=tpl["gmask"][c].reshape(tpl["ntile"], P, NG),
            ident=ident,
        ))
    import os
    trace = os.environ.get("BASS_KERNEL_TRACE", "0") == "1"
    if trace:
        sys.path.insert(0, "/root/problem/work")
        try:
            import axon_trace_patch  # noqa
        except Exception:
            trace = False
    res = run_bass_kernel_spmd(nc, in_maps, core_ids=list(range(NC)), trace=trace)
    kernel.last_exec_ns = res.exec_time_ns

    gsum = np.zeros((NG, D), dtype=np.float64)
    gmax = np.full((NG, D), -np.inf)
    for c in range(NC):
        gsum += res.results[c]["gsum"].astype(np.float64)
        gm = res.results[c]["gmax"].astype(np.float64).T   # [NG? no: [P,NG]->T = [NG,P]]
        pres = tpl["cells"][c] > 0
        gmax[pres] = np.maximum(gmax[pres], gm[pres])
    out = np.concatenate([gmax, gsum], axis=1).astype(np.float32)
    return (np.round(out * 1000.0) / 1000.0).astype(np.float32)
